# revision 5
# baseline (speedup 1.0000x reference)
"""Trainium2 Bass kernel for nn_NumAttention (sparse_attention).

Reference computation (per batch b, head i):
    k     = blockmix(x_cat, softmax(W_K)[i])            # [P, DH]
    xq    = blockmix(x_cat, softmax(W_Q)[i])            # [P, DH]
    q     = xq @ softmax(W_pred)[i]                     # [P, DH]
    v     = x_num @ softmax(W_V)[i]                     # [P]
    z[qp] = sum_{p<=qp} v[p] * (k[p] . q[qp])           # causal, no softmax

Key restructuring: attention here is softmax-free with scalar values, so it
is *linear*:  z[qp] = xq[qp] . S[qp]  with  S = cumsum_p(v[p] * ktilde[p,:])
where ktilde = k @ pp^T folds the W_pred mix into the k side.  The O(P^2)
score matrix is never materialized; per-core device work is one
[P,512]x[512,256] bf16 mix matmul (fp32 accumulate) plus a chunked cumsum.

The chunked cumsum: per 128-position chunk pair, S = triT_incl @ vk with the
inter-chunk carry added by a selector matmul that broadcasts the exclusive
pair prefix Tex[j] over all positions of the pair.

Scheduling (the point of this version):
 - The PE clock needs ~4-5us of *continuous* busy to ramp 0.65->2.4GHz, so
   warm-up dummies run back-to-back directly into the real mix stream.
 - DMA: W is split across both DGE rings at the head so the first mix chunk
   is gated only by the first x slice; all small constants ride in two
   packed transfers (one HWDGE descriptor-gen per ring instead of six).
 - The per-pair column sums (T2) and the triangular matmuls of pass 2 are
   interleaved into the mix stream two chunks behind the producing vk drain,
   so after the last mix chunk only T2[7] + the tiny prefix + 8 selector
   matmuls + drains remain.
 - Pass-2 drains alternate between a DVE-direct path (mult straight from
   PSUM) and an ACT-copy path so the tail burst is split across engines.
 - v, z are bf16 on the wire (half the DMA bytes; z descriptor count halves).

Sharding: 8 cores = 4 batches x 2 head-groups (4 heads each).  Host ships
x_cat[b] pre-transposed to feature-major bf16, the per-head effective mix
weights, and host-computed v (x_num @ pv, 8 MFLOP).
"""

import numpy as np
import ml_dtypes

import concourse.bacc as bacc
import concourse.mybir as mybir
import concourse.tile as tile
from concourse.bass_utils import run_bass_kernel_spmd

B, P, DC, DN, H, DH = 4, 2048, 512, 64, 8, 64
NV = DC // DH
CH = 128          # positions per chunk
NCH = P // CH     # 16 chunks
NPR = NCH // 2    # 8 chunk pairs
HPC = 4           # heads per core
FH = HPC * DH     # 256 = stacked-head free width
FH2 = 2 * FH      # 512 = pair width
NCORES = 8
KC = DC // CH     # 4 feature K-chunks
NWARM = 7         # PE warm-up dummy matmuls

# packed-constant column offsets (cpa: 128-partition, cpb: 8-partition)
CPA_V = 0                      # v, bf16, [CH, NCH*HPC] pos-chunk-major
CPA_TRIT = CPA_V + NCH * HPC   # trit [CH, CH]
CPA_ONEH = CPA_TRIT + CH       # oneh [CH, NPR*NPR]
CPA_W = CPA_ONEH + NPR * NPR
CPB_STRT = 0                   # strict upper tri [NPR, NPR]
CPB_STRTI = CPB_STRT + NPR     # inclusive upper tri
CPB_SEL = CPB_STRTI + NPR      # sel [NPR, NPR*CH]
CPB_W = CPB_SEL + NPR * CH

_BF16 = ml_dtypes.bfloat16

_cache = {}


def _softmax(x, axis=-1):
    e = np.exp(x - x.max(axis=axis, keepdims=True))
    return e / e.sum(axis=axis, keepdims=True)


def _build_program():
    nc = bacc.Bacc()
    f32 = mybir.dt.float32
    bf16 = mybir.dt.bfloat16
    mult = mybir.AluOpType.mult
    add = mybir.AluOpType.add

    w_d = nc.dram_tensor("w", [CH, KC, FH2], bf16, kind="ExternalInput")
    xct_d = nc.dram_tensor("xct", [8, CH, KC, P // 8], bf16, kind="ExternalInput")
    cpa_d = nc.dram_tensor("cpa", [CH, CPA_W], bf16, kind="ExternalInput")
    cpb_d = nc.dram_tensor("cpb", [NPR, CPB_W], bf16, kind="ExternalInput")
    z_d = nc.dram_tensor("z", [CH, NCH * HPC], bf16, kind="ExternalOutput")

    with tile.TileContext(nc) as tc:
        with (
            tc.tile_pool(name="persist", bufs=1) as pers,
            tc.tile_pool(name="work", bufs=4) as work,
            tc.tile_pool(name="mixp", bufs=2, space="PSUM") as mixp,
            tc.tile_pool(name="spre", bufs=4, space="PSUM") as spre,
            tc.tile_pool(name="pt2", bufs=1, space="PSUM") as pt2,
            tc.tile_pool(name="ptexw", bufs=1, space="PSUM") as ptexw,
        ):
            w_sb = pers.tile([CH, KC, FH2], bf16, tag="w_sb")
            xcT = pers.tile([CH, 8, KC, P // 8], bf16, tag="xcT")
            cpa = pers.tile([CH, CPA_W], bf16, tag="cpa")
            cpb = pers.tile([NPR, CPB_W], bf16, tag="cpb")
            vk_sb = pers.tile([CH, NCH, FH], bf16, tag="vk_sb")
            q_sb = pers.tile([CH, NCH, FH], bf16, tag="q_sb")
            t2_sb = pers.tile([NPR, FH2], bf16, tag="t2_sb")
            texw_sb = pers.tile([NPR, FH2], bf16, tag="texw_sb")
            z_sb = pers.tile([CH, NCH * HPC], bf16, tag="z_sb")
            dumw = pers.tile([CH, FH2], bf16, tag="dumw")

            trit = cpa[:, CPA_TRIT : CPA_TRIT + CH]
            strt = cpb[:, CPB_STRT : CPB_STRT + NPR]
            strti = cpb[:, CPB_STRTI : CPB_STRTI + NPR]

            # ---- PE warm-up: back-to-back dummy matmuls ramp the PE clock
            # (0.65 -> 2.4GHz needs ~4-5us of continuous busy) while the
            # first DMA transfers are still in flight.
            nc.gpsimd.memset(dumw[:], 0.0)
            warm_tiles = []
            for i in range(NWARM):
                pw = mixp.tile([CH, FH2], f32, tag="psum_mix")
                warm_tiles.append(pw)
                nc.tensor.matmul(
                    pw[:], dumw[:, 0:CH], dumw[:], start=True, stop=True
                )

            # ---- loads. Two HWDGE rings (sync / scalar) drain their
            # transfers in order; descriptor generation is ~0.6us per
            # transfer, so W is split across both rings at the head and the
            # small constants ride in two packed transfers.
            nc.sync.dma_start(out=w_sb[:, 0:2, :], in_=w_d[:, 0:2, :])
            nc.scalar.dma_start(out=w_sb[:, 2:4, :], in_=w_d[:, 2:4, :])
            nc.sync.dma_start(out=xcT[:, 0], in_=xct_d[0])
            nc.scalar.dma_start(out=xcT[:, 1], in_=xct_d[1])
            nc.scalar.dma_start(out=cpa[:], in_=cpa_d[:])
            nc.sync.dma_start(out=xcT[:, 2], in_=xct_d[2])
            nc.scalar.dma_start(out=xcT[:, 3], in_=xct_d[3])
            nc.sync.dma_start(out=xcT[:, 4], in_=xct_d[4])
            nc.scalar.dma_start(out=xcT[:, 5], in_=xct_d[5])
            nc.sync.dma_start(out=xcT[:, 6], in_=xct_d[6])
            nc.scalar.dma_start(out=xcT[:, 7], in_=xct_d[7])
            nc.scalar.dma_start(out=cpb[:], in_=cpb_d[:])

            psum_t2 = pt2.tile([NPR, FH2], f32, tag="psum_t2")
            psum_texw = ptexw.tile([NPR, FH2], f32, tag="psum_texw")
            s_tiles = [None] * NPR

            def vk_pair(j):
                return vk_sb[:, 2 * j : 2 * j + 2, :].rearrange("p c f -> p (c f)")

            # ---- pass 1: mix matmuls -> vk (bf16), q (bf16).  The per-pair
            # column sums (T2) and the triangular half of pass 2 are
            # interleaved two chunks behind the vk drains so the PE never
            # waits on the DVE and the tail stays short.
            for c in range(NCH):
                psum_mix = mixp.tile([CH, FH2], f32, tag="psum_mix")
                for kc in range(KC):
                    nc.tensor.matmul(
                        psum_mix[:],
                        xcT[:, c // 2, kc, (c % 2) * CH : (c % 2 + 1) * CH],
                        w_sb[:, kc, :],
                        start=(kc == 0),
                        stop=(kc == KC - 1),
                    )
                # vk[p, i, h] = ktilde[p, i, h] * v[p, i]
                nc.vector.tensor_tensor(
                    out=vk_sb[:, c, :].rearrange("p (i h) -> p i h", h=DH),
                    in0=psum_mix[:, 0:FH].rearrange("p (i h) -> p i h", h=DH),
                    in1=cpa[:, CPA_V + c * HPC : CPA_V + (c + 1) * HPC]
                    .unsqueeze(2)
                    .broadcast_to([CH, HPC, DH]),
                    op=mult,
                )
                nc.scalar.copy(q_sb[:, c, :], psum_mix[:, FH:FH2])

                if c >= 3 and (c - 3) % 2 == 0:
                    j = (c - 3) // 2
                    # T2[j] = [colsum(vk_{2j}) | colsum(vk_{2j+1})]
                    nc.tensor.matmul(
                        psum_t2[:],
                        cpa[:, CPA_ONEH + j * NPR : CPA_ONEH + (j + 1) * NPR],
                        vk_pair(j),
                        start=(j == 0),
                        stop=False,
                    )
                if c >= 5 and (c - 5) % 2 == 0:
                    jj = (c - 5) // 2
                    s_tiles[jj] = spre.tile(
                        [CH, FH2], f32, tag="psum_s", name="psum_s"
                    )
                    nc.tensor.matmul(
                        s_tiles[jj][:], trit, vk_pair(jj), start=True, stop=False
                    )

            # last pair sum (vk pair 7 drains ~0.4us after the final mix
            # matmul; two dummies keep the PE clock from idling down)
            for i in range(2):
                pw = mixp.tile([CH, FH2], f32, tag="psum_mix")
                nc.tensor.matmul(
                    pw[:], dumw[:, 0:CH], dumw[:], start=True, stop=True
                )
            nc.tensor.matmul(
                psum_t2[:],
                cpa[:, CPA_ONEH + 7 * NPR : CPA_ONEH + 8 * NPR],
                vk_pair(7),
                start=False,
                stop=True,
            )

            # ---- pair-level exclusive prefix:
            #   Tex[2j]   = sum_{a<j} (T2L+T2R)[a]  = strt@t2L + strt@t2R
            #   Tex[2j+1] = Tex[2j] + T2L[j]        = strtI@t2L + strt@t2R
            nc.vector.tensor_copy(t2_sb[:], psum_t2[:])
            nc.tensor.matmul(
                psum_texw[:, 0:FH], strt, t2_sb[:, 0:FH], start=True, stop=False
            )
            nc.tensor.matmul(
                psum_texw[:, 0:FH], strt, t2_sb[:, FH:FH2], start=False, stop=True
            )
            nc.tensor.matmul(
                psum_texw[:, FH:FH2], strti, t2_sb[:, 0:FH], start=True, stop=False
            )
            nc.tensor.matmul(
                psum_texw[:, FH:FH2], strt, t2_sb[:, FH:FH2], start=False, stop=True
            )
            nc.vector.tensor_copy(texw_sb[:], psum_texw[:])

            # ---- pass 2: close each pair with the carry-broadcast selector
            # matmul, then drain z = rowsum(q * S).  Drains alternate between
            # a DVE-direct path and an ACT-copy path to split the tail burst.
            for j in range(NPR):
                if s_tiles[j] is None:
                    s_tiles[j] = spre.tile(
                        [CH, FH2], f32, tag="psum_s", name="psum_s"
                    )
                    nc.tensor.matmul(
                        s_tiles[j][:], trit, vk_pair(j), start=True, stop=False
                    )
                nc.tensor.matmul(
                    s_tiles[j][:],
                    cpb[:, CPB_SEL + j * CH : CPB_SEL + (j + 1) * CH],
                    texw_sb[:],
                    start=False,
                    stop=True,
                )
                prod = work.tile([CH, FH2], bf16, tag="prod")
                if j % 2 == 0:
                    nc.vector.tensor_tensor(
                        out=prod[:],
                        in0=q_sb[:, 2 * j : 2 * j + 2, :].rearrange(
                            "p c f -> p (c f)"
                        ),
                        in1=s_tiles[j][:],
                        op=mult,
                    )
                else:
                    s_sb = work.tile([CH, FH2], bf16, tag="s_sb")
                    nc.scalar.copy(s_sb[:], s_tiles[j][:])
                    nc.vector.tensor_tensor(
                        out=prod[:],
                        in0=q_sb[:, 2 * j : 2 * j + 2, :].rearrange(
                            "p c f -> p (c f)"
                        ),
                        in1=s_sb[:],
                        op=mult,
                    )
                with nc.allow_low_precision(
                    reason="z reduce in bf16; rel-err budget is 2e-2"
                ):
                    nc.vector.tensor_reduce(
                        out=z_sb[:, 2 * j * HPC : (2 * j + 2) * HPC],
                        in_=prod[:].rearrange("p (ci h) -> p ci h", h=DH),
                        axis=mybir.AxisListType.X,
                        op=add,
                    )
                if j == 3:
                    nc.sync.dma_start(
                        out=z_d[:, 0 : 8 * HPC], in_=z_sb[:, 0 : 8 * HPC]
                    )

            nc.sync.dma_start(out=z_d[:, 8 * HPC :], in_=z_sb[:, 8 * HPC :])

    nc.finalize()
    return nc


def _host_inputs(x_cat, x_num, W_K, W_Q, W_pred, W_V):
    """Per-core input maps. Core c = batch (c//2), head-group (c%2)."""
    pk = _softmax(W_K.astype(np.float64)).astype(np.float32)
    pq = _softmax(W_Q.astype(np.float64)).astype(np.float32)
    pp = _softmax(W_pred.astype(np.float64)).astype(np.float32)
    pv = _softmax(W_V.astype(np.float64)).astype(np.float32)

    trit = np.triu(np.ones((CH, CH), np.float32))
    oneh = np.zeros((CH, NPR, NPR), np.float32)
    oneh[:, np.arange(NPR), np.arange(NPR)] = 1.0
    oneh = oneh.reshape(CH, NPR * NPR)
    strt = np.triu(np.ones((NPR, NPR), np.float32), k=1)
    strti = np.triu(np.ones((NPR, NPR), np.float32), k=0)
    sel = np.zeros((NPR, NPR, CH), np.float32)
    sel[np.arange(NPR), np.arange(NPR), :] = 1.0
    sel = sel.reshape(NPR, NPR * CH)

    cpb = np.concatenate([strt, strti, sel], axis=1).astype(_BF16)

    eye = np.eye(DH, dtype=np.float32)
    v_full = np.einsum("bpd,id->bpi", x_num, pv)  # [B, P, H] fp32, host-side

    in_maps = []
    for core in range(NCORES):
        b, hg = core // 2, core % 2
        heads = range(hg * HPC, (hg + 1) * HPC)
        W = np.zeros((DC, FH2), np.float32)
        for j, i in enumerate(heads):
            # ktilde cols: W[(v,g), j*64+h] = pk[i,v] * pp[i,h,g]
            W[:, j * DH : (j + 1) * DH] = (
                pk[i][:, None, None] * pp[i].T[None, :, :]
            ).reshape(DC, DH)
            # xq cols: W[(v,h), FH + j*64+h'] = pq[i,v] * delta(h,h')
            W[:, FH + j * DH : FH + (j + 1) * DH] = np.kron(pq[i][:, None], eye)
        # per-partition contiguous slice blocks
        xq8 = x_cat[b].T.reshape(KC, CH, 8, P // 8).transpose(2, 1, 0, 3)
        wq = W.reshape(KC, CH, FH2).transpose(1, 0, 2)
        # v in device layout [p, (chunk, head)]
        v_core = v_full[b][:, hg * HPC : (hg + 1) * HPC]  # [P, HPC]
        v_dev = (
            v_core.reshape(NCH, CH, HPC).transpose(1, 0, 2).reshape(CH, NCH * HPC)
        )
        cpa = np.concatenate([v_dev, trit, oneh], axis=1).astype(_BF16)
        in_maps.append(
            {
                "xct": np.ascontiguousarray(xq8).astype(_BF16),
                "w": np.ascontiguousarray(wq).astype(_BF16),
                "cpa": np.ascontiguousarray(cpa),
                "cpb": np.ascontiguousarray(cpb),
            }
        )
    return in_maps


def _run(inputs, **spmd_kwargs):
    if "nc" not in _cache:
        _cache["nc"] = _build_program()
    nc = _cache["nc"]

    in_maps = _host_inputs(**inputs)
    res = run_bass_kernel_spmd(nc, in_maps, list(range(NCORES)), **spmd_kwargs)

    out = np.zeros((B, P, H), np.float32)
    for core in range(NCORES):
        b, hg = core // 2, core % 2
        z = res.results[core]["z"].astype(np.float32)  # [128, NCH*HPC]
        z = z.reshape(CH, NCH, HPC).transpose(1, 0, 2).reshape(P, HPC)
        out[b, :, hg * HPC : (hg + 1) * HPC] = z
    return out, res


def kernel(x_cat, x_num, W_K, W_Q, W_pred, W_V):
    out, _ = _run(
        dict(x_cat=x_cat, x_num=x_num, W_K=W_K, W_Q=W_Q, W_pred=W_pred, W_V=W_V)
    )
    return out


# revision 10
# speedup vs baseline: 1.2755x; 1.2755x over previous
"""Trainium2 Bass kernel for nn_NumAttention (sparse_attention).

Reference computation (per batch b, head i):
    k     = blockmix(x_cat, softmax(W_K)[i])            # [P, DH]
    xq    = blockmix(x_cat, softmax(W_Q)[i])            # [P, DH]
    q     = xq @ softmax(W_pred)[i]                     # [P, DH]
    v     = x_num @ softmax(W_V)[i]                     # [P]
    z[qp] = sum_{p<=qp} v[p] * (k[p] . q[qp])           # causal, no softmax

Key restructuring: attention here is softmax-free with scalar values, so it
is *linear*:  z[qp] = xq[qp] . S[qp]  with  S = cumsum_p(v[p] * ktilde[p,:])
where ktilde = k @ pp^T folds the W_pred mix into the k side.  The O(P^2)
score matrix is never materialized; per-core device work is one
[P,512]x[512,512] bf16 mix matmul (fp32 accumulate) plus a chunked cumsum.

The chunked cumsum: per 128-position chunk pair, S = triT_incl @ vk with the
inter-chunk carry added by a selector matmul that broadcasts the exclusive
pair prefix Tex[j] over all positions of the pair.

Scheduling (the point of this version):
 - The PE clock needs ~4-5us of *continuous* busy to ramp 0.65->2.4GHz, so
   warm-up dummies run back-to-back directly into the real mix stream.
 - DMA: x slice 0 leads the sync ring, W is split across the scalar and
   vector DGE rings, so the first mix chunk is gated only by ~0.75MB of
   front bytes; all small constants ride in two packed transfers.
 - The pair column sums (T2) accumulate in TWO psum groups (pairs 0-3 and
   4-7).  Group A closes mid-stream, so its prefix, selector matmuls and
   z drains all run interleaved with the remaining mix chunks; only pairs
   4-7 remain after the last mix matmul.
 - Pass-2 drains: ACT copies S to sbuf bf16, DVE runs the 2x-mode multiply
   and the rowsum; the tiny prefix-table drains ride the idle GPSIMD.
 - v, z are bf16 on the wire (half the DMA bytes; z descriptor count halves).

Sharding: 8 cores = 4 batches x 2 head-groups (4 heads each).  Host ships
x_cat[b] pre-transposed to feature-major bf16, the per-head effective mix
weights, and host-computed v (x_num @ pv, 8 MFLOP).
"""

import numpy as np
import ml_dtypes

import concourse.bacc as bacc
import concourse.mybir as mybir
import concourse.tile as tile
from concourse.bass_utils import run_bass_kernel_spmd

B, P, DC, DN, H, DH = 4, 2048, 512, 64, 8, 64
NV = DC // DH
CH = 128          # positions per chunk
NCH = P // CH     # 16 chunks
NPR = NCH // 2    # 8 chunk pairs
NG = NPR // 2     # 4 pairs per T2 group
HPC = 4           # heads per core
FH = HPC * DH     # 256 = stacked-head free width
FH2 = 2 * FH      # 512 = pair width
NCORES = 8
KC = DC // CH     # 4 feature K-chunks
NWARM = 10        # PE warm-up dummy matmuls

# packed-constant column offsets (cpa: 128-partition, cpb: 4-partition rows)
CPA_V = 0                      # v, bf16, [CH, NCH*HPC] pos-chunk-major
CPA_TRIT = CPA_V + NCH * HPC   # trit [CH, CH]
CPA_ONEH = CPA_TRIT + CH       # oneh [CH, NG*NG] (col (j,m) = (m==j))
CPA_W = CPA_ONEH + NG * NG
CPB_STRT = 0                   # strict upper tri [NG, NG]
CPB_STRTI = CPB_STRT + NG      # inclusive upper tri
CPB_ONES = CPB_STRTI + NG      # all-ones [NG, NG]
CPB_SEL = CPB_ONES + NG        # sel [NG, NG*CH] (sel[k, jj*CH+p] = (k==jj))
CPB_W = CPB_SEL + NG * CH

_BF16 = ml_dtypes.bfloat16

_cache = {}


def _softmax(x, axis=-1):
    e = np.exp(x - x.max(axis=axis, keepdims=True))
    return e / e.sum(axis=axis, keepdims=True)


def _build_program():
    nc = bacc.Bacc()
    f32 = mybir.dt.float32
    bf16 = mybir.dt.bfloat16
    mult = mybir.AluOpType.mult
    add = mybir.AluOpType.add

    w_d = nc.dram_tensor("w", [CH, KC, FH2], bf16, kind="ExternalInput")
    xct_d = nc.dram_tensor("xct", [8, CH, KC, P // 8], bf16, kind="ExternalInput")
    cpa_d = nc.dram_tensor("cpa", [CH, CPA_W], bf16, kind="ExternalInput")
    cpb_d = nc.dram_tensor("cpb", [NG, CPB_W], bf16, kind="ExternalInput")
    z_d = nc.dram_tensor("z", [CH, NCH * HPC], bf16, kind="ExternalOutput")

    with tile.TileContext(nc) as tc:
        with (
            tc.tile_pool(name="persist", bufs=1) as pers,
            tc.tile_pool(name="work", bufs=4) as work,
            tc.tile_pool(name="mixp", bufs=3, space="PSUM") as mixp,
            tc.tile_pool(name="spre", bufs=3, space="PSUM") as spre,
            tc.tile_pool(name="pt2", bufs=1, space="PSUM") as pt2,
            tc.tile_pool(name="ptexw", bufs=1, space="PSUM") as ptexw,
        ):
            w_sb = pers.tile([CH, KC, FH2], bf16, tag="w_sb")
            xcT = pers.tile([CH, 8, KC, P // 8], bf16, tag="xcT")
            cpa = pers.tile([CH, CPA_W], bf16, tag="cpa")
            cpb = pers.tile([NG, CPB_W], bf16, tag="cpb")
            vk_sb = pers.tile([CH, NCH, FH], bf16, tag="vk_sb")
            q_sb = pers.tile([CH, NCH, FH], bf16, tag="q_sb")
            t2a_sb = pers.tile([NG, FH2], bf16, tag="t2a_sb")
            t2b_sb = pers.tile([NG, FH2], bf16, tag="t2b_sb")
            texwa_sb = pers.tile([NG, FH2], bf16, tag="texwa_sb")
            texwb_sb = pers.tile([NG, FH2], bf16, tag="texwb_sb")
            z_sb = pers.tile([CH, NCH * HPC], bf16, tag="z_sb")
            dumw = pers.tile([CH, FH2], bf16, tag="dumw")

            trit = cpa[:, CPA_TRIT : CPA_TRIT + CH]
            strt4 = cpb[:, CPB_STRT : CPB_STRT + NG]
            strti4 = cpb[:, CPB_STRTI : CPB_STRTI + NG]
            ones4 = cpb[:, CPB_ONES : CPB_ONES + NG]

            def oneh(j):
                return cpa[:, CPA_ONEH + j * NG : CPA_ONEH + (j + 1) * NG]

            def sel(jj):
                return cpb[:, CPB_SEL + jj * CH : CPB_SEL + (jj + 1) * CH]

            # ---- PE warm-up: back-to-back dummy matmuls ramp the PE clock
            # (0.65 -> 2.4GHz needs ~4-5us of continuous busy) while the
            # first DMA transfers are still in flight.
            nc.gpsimd.memset(dumw[:], 0.0)
            for i in range(NWARM):
                pw = mixp.tile([CH, FH2], f32, tag="psum_mix", name="psum_mix")
                nc.tensor.matmul(
                    pw[:], dumw[:, 0:CH], dumw[:], start=True, stop=True
                )

            # ---- loads across three DGE rings.
            nc.sync.dma_start(out=xcT[:, 0], in_=xct_d[0])
            nc.scalar.dma_start(out=w_sb[:, 0:2, :], in_=w_d[:, 0:2, :])
            nc.gpsimd.dma_start(out=w_sb[:, 2:4, :], in_=w_d[:, 2:4, :])
            nc.scalar.dma_start(out=cpa[:], in_=cpa_d[:])
            nc.sync.dma_start(out=xcT[:, 2], in_=xct_d[2])
            nc.scalar.dma_start(out=xcT[:, 1], in_=xct_d[1])
            nc.sync.dma_start(out=xcT[:, 4], in_=xct_d[4])
            nc.scalar.dma_start(out=xcT[:, 3], in_=xct_d[3])
            nc.scalar.dma_start(out=cpb[:], in_=cpb_d[:])
            nc.sync.dma_start(out=xcT[:, 6], in_=xct_d[6])
            nc.scalar.dma_start(out=xcT[:, 5], in_=xct_d[5])
            nc.scalar.dma_start(out=xcT[:, 7], in_=xct_d[7])

            psum_t2a = pt2.tile([NG, FH2], f32, tag="psum_t2", name="psum_t2a")
            psum_texwa = ptexw.tile(
                [NG, FH2], f32, tag="psum_texw", name="psum_texwa"
            )
            s_tiles = [None] * NPR
            psum_t2b = None

            def vk_pair(j):
                return vk_sb[:, 2 * j : 2 * j + 2, :].rearrange("p c f -> p (c f)")

            def q_pair(j):
                return q_sb[:, 2 * j : 2 * j + 2, :].rearrange("p c f -> p (c f)")

            def trit_part(j):
                t = spre.tile([CH, FH2], f32, tag="psum_s", name="psum_s")
                s_tiles[j] = t
                nc.tensor.matmul(t[:], trit, vk_pair(j), start=True, stop=False)

            def sel_part(j):
                # close pair j: add the carry-broadcast of Tex[j]
                texw = texwa_sb if j < NG else texwb_sb
                nc.tensor.matmul(
                    s_tiles[j][:], sel(j % NG), texw[:], start=False, stop=True
                )

            def drain_pair(j):
                s_sb = work.tile([CH, FH2], bf16, tag="s_sb", name="s_sb")
                nc.scalar.copy(s_sb[:], s_tiles[j][:])
                prod = work.tile([CH, FH2], bf16, tag="prod", name="prod")
                nc.vector.tensor_tensor(
                    out=prod[:], in0=q_pair(j), in1=s_sb[:], op=mult
                )
                with nc.allow_low_precision(
                    reason="z reduce in bf16; rel-err budget is 2e-2"
                ):
                    nc.vector.tensor_reduce(
                        out=z_sb[:, 2 * j * HPC : (2 * j + 2) * HPC],
                        in_=prod[:].rearrange("p (ci h) -> p ci h", h=DH),
                        axis=mybir.AxisListType.X,
                        op=add,
                    )

            # ---- pass 1 with interleaved pass-2 work.  T2 group A (pairs
            # 0-3) closes at chunk 9; its prefix + selector matmuls + drains
            # run during chunks 10-15.
            for c in range(NCH):
                psum_mix = mixp.tile([CH, FH2], f32, tag="psum_mix", name="psum_mix")
                for kc in range(KC):
                    nc.tensor.matmul(
                        psum_mix[:],
                        xcT[:, c // 2, kc, (c % 2) * CH : (c % 2 + 1) * CH],
                        w_sb[:, kc, :],
                        start=(kc == 0),
                        stop=(kc == KC - 1),
                    )
                # vk[p, i, h] = ktilde[p, i, h] * v[p, i]
                nc.vector.tensor_tensor(
                    out=vk_sb[:, c, :].rearrange("p (i h) -> p i h", h=DH),
                    in0=psum_mix[:, 0:FH].rearrange("p (i h) -> p i h", h=DH),
                    in1=cpa[:, CPA_V + c * HPC : CPA_V + (c + 1) * HPC]
                    .unsqueeze(2)
                    .broadcast_to([CH, HPC, DH]),
                    op=mult,
                )
                nc.scalar.copy(q_sb[:, c, :], psum_mix[:, FH:FH2])

                if c in (3, 5, 7, 9):
                    j = (c - 3) // 2
                    nc.tensor.matmul(
                        psum_t2a[:],
                        oneh(j),
                        vk_pair(j),
                        start=(j == 0),
                        stop=(j == NG - 1),
                    )
                if c == 5:
                    trit_part(0)
                if c == 7:
                    trit_part(1)
                if c == 9:
                    nc.vector.tensor_copy(t2a_sb[:], psum_t2a[:])
                if c == 10:
                    trit_part(2)
                    # prefix A: Tex[2j] = strt4@(t2aL+t2aR); Tex[2j+1] += t2aL
                    nc.tensor.matmul(
                        psum_texwa[:, 0:FH], strt4, t2a_sb[:, 0:FH],
                        start=True, stop=False,
                    )
                    nc.tensor.matmul(
                        psum_texwa[:, 0:FH], strt4, t2a_sb[:, FH:FH2],
                        start=False, stop=True,
                    )
                    nc.tensor.matmul(
                        psum_texwa[:, FH:FH2], strti4, t2a_sb[:, 0:FH],
                        start=True, stop=False,
                    )
                    nc.tensor.matmul(
                        psum_texwa[:, FH:FH2], strt4, t2a_sb[:, FH:FH2],
                        start=False, stop=True,
                    )
                    nc.scalar.copy(texwa_sb[:], psum_texwa[:])
                if c == 12:
                    sel_part(0)
                    drain_pair(0)
                if c == 13:
                    sel_part(1)
                    trit_part(3)
                    drain_pair(1)
                    psum_t2b = pt2.tile(
                        [NG, FH2], f32, tag="psum_t2", name="psum_t2b"
                    )
                    nc.tensor.matmul(
                        psum_t2b[:], oneh(0), vk_pair(4), start=True, stop=False
                    )
                if c == 14:
                    sel_part(2)
                    trit_part(4)
                    drain_pair(2)
                    nc.tensor.matmul(
                        psum_t2b[:], oneh(1), vk_pair(5), start=False, stop=False
                    )
                if c == 15:
                    sel_part(3)
                    trit_part(5)
                    drain_pair(3)
                    nc.tensor.matmul(
                        psum_t2b[:], oneh(2), vk_pair(6), start=False, stop=False
                    )

            # first half of z goes out while the tail computes
            nc.sync.dma_start(
                out=z_d[:, 0 : 2 * NG * HPC], in_=z_sb[:, 0 : 2 * NG * HPC]
            )

            # last pair sums need the vk drains of chunks 13-15; two dummies
            # keep the PE clock up during that short wait
            for i in range(2):
                pw = mixp.tile([CH, FH2], f32, tag="psum_mix", name="psum_mix")
                nc.tensor.matmul(
                    pw[:], dumw[:, 0:CH], dumw[:], start=True, stop=True
                )
            nc.tensor.matmul(
                psum_t2b[:], oneh(3), vk_pair(7), start=False, stop=True
            )

            # ---- prefix B: Tex over pairs 4-7 = (sum of all group A) +
            # within-B exclusive prefix
            nc.vector.tensor_copy(t2b_sb[:], psum_t2b[:])
            psum_texwb = ptexw.tile(
                [NG, FH2], f32, tag="psum_texw", name="psum_texwb"
            )
            for half, lo in ((0, 0), (1, FH)):
                first = strt4 if half == 0 else strti4
                nc.tensor.matmul(
                    psum_texwb[:, lo : lo + FH], ones4, t2a_sb[:, 0:FH],
                    start=True, stop=False,
                )
                nc.tensor.matmul(
                    psum_texwb[:, lo : lo + FH], ones4, t2a_sb[:, FH:FH2],
                    start=False, stop=False,
                )
                nc.tensor.matmul(
                    psum_texwb[:, lo : lo + FH], first, t2b_sb[:, 0:FH],
                    start=False, stop=False,
                )
                nc.tensor.matmul(
                    psum_texwb[:, lo : lo + FH], strt4, t2b_sb[:, FH:FH2],
                    start=False, stop=True,
                )
            nc.scalar.copy(texwb_sb[:], psum_texwb[:])

            # ---- pass 2 tail: pairs 4-7
            trit_part(6)
            trit_part(7)
            for j in range(NG, NPR):
                sel_part(j)
                drain_pair(j)

            nc.sync.dma_start(
                out=z_d[:, 2 * NG * HPC :], in_=z_sb[:, 2 * NG * HPC :]
            )

    nc.finalize()
    return nc


def _host_inputs(x_cat, x_num, W_K, W_Q, W_pred, W_V):
    """Per-core input maps. Core c = batch (c//2), head-group (c%2)."""
    pk = _softmax(W_K.astype(np.float64)).astype(np.float32)
    pq = _softmax(W_Q.astype(np.float64)).astype(np.float32)
    pp = _softmax(W_pred.astype(np.float64)).astype(np.float32)
    pv = _softmax(W_V.astype(np.float64)).astype(np.float32)

    trit = np.triu(np.ones((CH, CH), np.float32))
    oneh = np.zeros((CH, NG, NG), np.float32)
    oneh[:, np.arange(NG), np.arange(NG)] = 1.0
    oneh = oneh.reshape(CH, NG * NG)
    strt4 = np.triu(np.ones((NG, NG), np.float32), k=1)
    strti4 = np.triu(np.ones((NG, NG), np.float32), k=0)
    ones4 = np.ones((NG, NG), np.float32)
    sel = np.zeros((NG, NG, CH), np.float32)
    sel[np.arange(NG), np.arange(NG), :] = 1.0
    sel = sel.reshape(NG, NG * CH)

    cpb = np.concatenate([strt4, strti4, ones4, sel], axis=1).astype(_BF16)

    eye = np.eye(DH, dtype=np.float32)
    v_full = np.einsum("bpd,id->bpi", x_num, pv)  # [B, P, H] fp32, host-side

    in_maps = []
    for core in range(NCORES):
        b, hg = core // 2, core % 2
        heads = range(hg * HPC, (hg + 1) * HPC)
        W = np.zeros((DC, FH2), np.float32)
        for j, i in enumerate(heads):
            # ktilde cols: W[(v,g), j*64+h] = pk[i,v] * pp[i,h,g]
            W[:, j * DH : (j + 1) * DH] = (
                pk[i][:, None, None] * pp[i].T[None, :, :]
            ).reshape(DC, DH)
            # xq cols: W[(v,h), FH + j*64+h'] = pq[i,v] * delta(h,h')
            W[:, FH + j * DH : FH + (j + 1) * DH] = np.kron(pq[i][:, None], eye)
        # per-partition contiguous slice blocks
        xq8 = x_cat[b].T.reshape(KC, CH, 8, P // 8).transpose(2, 1, 0, 3)
        wq = W.reshape(KC, CH, FH2).transpose(1, 0, 2)
        # v in device layout [p, (chunk, head)]
        v_core = v_full[b][:, hg * HPC : (hg + 1) * HPC]  # [P, HPC]
        v_dev = (
            v_core.reshape(NCH, CH, HPC).transpose(1, 0, 2).reshape(CH, NCH * HPC)
        )
        cpa = np.concatenate([v_dev, trit, oneh], axis=1).astype(_BF16)
        in_maps.append(
            {
                "xct": np.ascontiguousarray(xq8).astype(_BF16),
                "w": np.ascontiguousarray(wq).astype(_BF16),
                "cpa": np.ascontiguousarray(cpa),
                "cpb": np.ascontiguousarray(cpb),
            }
        )
    return in_maps


def _run(inputs, **spmd_kwargs):
    if "nc" not in _cache:
        _cache["nc"] = _build_program()
    nc = _cache["nc"]

    in_maps = _host_inputs(**inputs)
    res = run_bass_kernel_spmd(nc, in_maps, list(range(NCORES)), **spmd_kwargs)

    out = np.zeros((B, P, H), np.float32)
    for core in range(NCORES):
        b, hg = core // 2, core % 2
        z = res.results[core]["z"].astype(np.float32)  # [128, NCH*HPC]
        z = z.reshape(CH, NCH, HPC).transpose(1, 0, 2).reshape(P, HPC)
        out[b, :, hg * HPC : (hg + 1) * HPC] = z
    return out, res


def kernel(x_cat, x_num, W_K, W_Q, W_pred, W_V):
    out, _ = _run(
        dict(x_cat=x_cat, x_num=x_num, W_K=W_K, W_Q=W_Q, W_pred=W_pred, W_V=W_V)
    )
    return out
